# revision 28
# baseline (speedup 1.0000x reference)
"""Trainium2 Bass kernel for nn_NonsharedPatchEmbed_86827058856432.

Computes, for a patchified [64, 3, 224, 224] fp32 image batch,

    out[b, p, o] = sum_i patches[b, p, i] * W[p, o, i] + bias[p, o]

with 196 independent Linear(768->768) layers (one per patch).

The problem is HBM-bound on W traffic (196*768*768 elements, each used
once per core under patch sharding), so precision is spent where the bytes
are: W is stored as fp8 e3m4 (4 mantissa bits) pre-scaled by x64 to center
its distribution in e3m4's narrow exponent range, and the activations are
bf16 pre-divided by 64 (exact, so the products come out unscaled). The PE
accepts mixed bf16 x fp8 operands; PSUM accumulation stays fp32. Measured
end-to-end relative error is 1.30e-2 against the 2e-2 gate, deterministic
for the fixed harness inputs.

Distribution: 196 = 8 * 24.5, so each core gets 24 full patches plus ONE
HALF of a shared patch (384 of its 768 outputs): patches 0-191 go 24 per
core, and patches 192-195 are split into 8 output-halves, one per core.
Every core therefore reads exactly 24.5/196 of W -- perfect balance, and
the per-core DMA-engine pool (16 engines x ~22.4 GB/s ~= 360 GB/s) is the
roofline. The half-patch job runs LAST so the post-last-W drain (compute +
PSUM copy + output write) is ~4x smaller than a full pair's.

Per-core kernel (column-tiled pairs):
  - 12 pairs of full patches, then the half-patch job.
  - For each pair, patch A owns PSUM partitions 0-63 (tile_position (0, 0)),
    patch B owns partitions 64-127 ((0, 64)). Each streams its own W^T as
    the moving operand; the batch activations (aT chunks, [128 x 64]) are
    the stationary operand.
  - The bias is applied with ONE K=4 matmul per output slice: a host-built
    0/1 selector as the stationary operand routes [hiA, loA, hiB, loB]
    bf16 bias terms to the right PSUM partition halves, starting each
    accumulation group exactly (hi+lo reconstructs fp32 bias to ~1e-7).
  - Each patch's W rides ONE ring as a single 4608B-per-partition DMA
    (2304B packets measured ~11% slower per engine); pairs alternate
    patches across the two rings. Acts ride SP, outputs/bias ride ACT,
    keeping both rings byte-even (~9.7 MB each).
  - W dma_starts are issued first in each iteration so the big stream's
    descriptors stay at the queue heads.

Layouts per core:
  aT  [128, 25, 6, 64]  bf16  aT[i, p, c, b] = patches[b, patch(p), 128c+i]/64
                              (p = 0..23 full patches, p = 24 half patch)
  Wt  [24, 128, 6, 768] fp8   Wt[p, i, c, o] = 64*W[patch(p), o, 128c+i] (e3m4)
  Wh  [128, 6, 384]     fp8   half-patch W slice, same scaling
  bhl2 [12, 4, 768]     bf16  per pair: [hiA, loA, hiB, loB] bias terms
  bh  [2, 384]          bf16  half-patch bias hi + lo
  sel [4, 128]          bf16  bias selector: sel[k, m] = ((k < 2) == (m < 64))
  outp [12, 128, 768]   bf16  pair j rows 0-63 -> patch 2j, 64-127 -> 2j+1
  outh [64, 384]        bf16  half-patch outputs
"""

import sys
import types

import numpy as np
import ml_dtypes


def _ensure_ntff_hook():
    """Make ``antenv.axon_hooks`` importable and install the NTFF profile
    hook. The image's read-only ``antenv`` package lacks ``axon_hooks``, so
    ``trn_boot`` silently skips hook installation and
    ``run_bass_kernel_spmd(trace=True)`` would either crash on the import or
    skip tracing. Harmless no-op when the real module exists; any failure
    degrades to no-trace rather than an error."""
    try:
        from antenv.axon_hooks import get_axon_ntff_profile_hook  # noqa: F401
    except ImportError:
        mod = types.ModuleType("antenv.axon_hooks")
        _hook = [None]
        mod.set_axon_ntff_profile_hook = lambda h: _hook.__setitem__(0, h)
        mod.get_axon_ntff_profile_hook = lambda: _hook[0]
        sys.modules["antenv.axon_hooks"] = mod
        try:
            import antenv

            antenv.axon_hooks = mod
        except ImportError:
            pass
    try:
        import antenv.axon_hooks as ah

        if ah.get_axon_ntff_profile_hook() is None:
            from trn_agent_boot.trn_boot import _ntff_profile_via_ctypes

            ah.set_axon_ntff_profile_hook(
                _ntff_profile_via_ctypes("/opt/axon/libaxon_pjrt.so")
            )
    except Exception:
        pass


_ensure_ntff_hook()

import concourse.tile as tile
import concourse.mybir as mybir
from concourse import bacc
from concourse.bass_utils import run_bass_kernel_spmd

f32 = mybir.dt.float32
bf16 = mybir.dt.bfloat16
f8 = mybir.dt.float8e3   # e3m4: 4 mantissa bits
WSCALE = 64.0            # W stored as W*64 in fp8; act pre-divided by 64 (both exact)

N_CORES = 8
B = 64            # batch
D = 768           # in/out feature dim
HD = 384          # half of D (half-patch job width)
NP = 196          # real patches
FPC = 24          # full patches per core (8*24 = 192)
NPAIR = FPC // 2  # 12 pairs
NCHUNK = 6        # 768 / 128 contraction chunks

LAST_RESULTS = None    # BassKernelResults of the most recent run (for test.py)

_NC_CACHE = {}


def _build():
    nc = bacc.Bacc()
    aT = nc.declare_dram_parameter("aT", [128, FPC + 1, NCHUNK, B], bf16, isOutput=False)
    Wt = nc.declare_dram_parameter("Wt", [FPC, 128, NCHUNK, D], f8, isOutput=False)
    Wh = nc.declare_dram_parameter("Wh", [128, NCHUNK, HD], f8, isOutput=False)
    bhl2 = nc.declare_dram_parameter("bhl2", [NPAIR, 4, D], bf16, isOutput=False)
    bh = nc.declare_dram_parameter("bh", [2, HD], bf16, isOutput=False)
    sel = nc.declare_dram_parameter("sel", [4, 2 * B], bf16, isOutput=False)
    outp = nc.declare_dram_parameter("outp", [NPAIR, 2 * B, D], bf16, isOutput=True)
    outh = nc.declare_dram_parameter("outh", [B, HD], bf16, isOutput=True)

    with tile.TileContext(nc) as tc:
        with (
            tc.tile_pool(name="const", bufs=1) as cpool,
            tc.tile_pool(name="a", bufs=6) as apool,
            tc.tile_pool(name="w", bufs=10) as wpool,
            tc.tile_pool(name="o", bufs=3) as opool,
            tc.tile_pool(name="ps", bufs=3, space="PSUM") as pspool,
            tc.tile_pool(name="psh", bufs=1, space="PSUM") as pshpool,
        ):
            # Selector stationary for the pair bias matmul: one K=4 matmul
            # covers both column tiles exactly (hi+lo bf16 terms per patch).
            # ones2[k, m] = 1 iff (k < 2) == (m < 64); its top-left [2, 64]
            # corner doubles as the all-ones operand for the half-patch job.
            # (Built on the host: engine memsets can't start at partition 2.)
            ones2 = cpool.tile([4, 2 * B], bf16)

            slices = [(0, 512), (512, 768)]

            def wtile(p, eng):
                # Each patch's W rides ONE ring as a single 4608B-per-partition
                # transfer (2304B packets run ~11% slower per DMA engine);
                # pairs alternate patches across the two rings, which keeps
                # both byte-even. A single ring caps near ~200 GB/s, so the
                # alternation — not one ring — carries the dominant traffic.
                t = wpool.tile([128, NCHUNK, D], f8, tag="wt")
                eng.dma_start(t[:], Wt[p])
                return t

            for j in range(NPAIR):
                p0, p1 = 2 * j, 2 * j + 1
                wt0 = wtile(p0, nc.sync)
                wt1 = wtile(p1, nc.scalar)
                if j == 0:
                    # Behind pair 0's W: the bias selector and the half-patch
                    # job's inputs. Prefetching the half job up front makes
                    # the final job start with everything resident (short
                    # drain); sequencing it after pair 0's W keeps the big
                    # stream's first descriptors at the queue heads.
                    nc.scalar.dma_start(ones2[:], sel[:, :])
                    wh = wpool.tile([128, NCHUNK, HD], f8, tag="wh")
                    nc.sync.dma_start(wh[:], Wh[:, :])
                    ah = apool.tile([128, 1, NCHUNK, B], bf16, tag="ah")
                    tbh = apool.tile([2, HD], bf16, tag="tbh")
                    nc.scalar.dma_start(ah[:], aT[:, FPC:FPC + 1])
                    nc.scalar.dma_start(tbh[:], bh[:, :])
                # Outputs exist only after compute, so they are inherently the
                # LAST bytes each queue moves; pinning them all to one ring
                # measured that ring finishing ~7.5us after the other.
                # Alternate outputs by pair parity (and acts oppositely, to
                # keep both rings byte-even) so the late tail splits evenly.
                at = apool.tile([128, 2, NCHUNK, B], bf16, tag="at")
                tb = apool.tile([4, D], bf16, tag="tb")
                eng_at = nc.sync if j % 2 == 0 else nc.scalar
                eng_at.dma_start(at[:], aT[:, p0:p0 + 2])
                nc.scalar.dma_start(tb[:], bhl2[j])
                a0 = at[:, 0]
                a1 = at[:, 1]

                pt = pspool.tile([2 * B, D], f32, tag="pt")
                for (o0, o1) in slices:
                    nc.tensor.matmul(
                        pt[:, o0:o1], ones2[:], tb[:, o0:o1],
                        start=True, stop=False,
                    )
                for c in range(NCHUNK):
                    # A's two output slices adjacent so the stationary
                    # operand only changes once per chunk per patch.
                    for (o0, o1) in slices:
                        nc.tensor.matmul(
                            pt[:B, o0:o1], a0[:, c, :], wt0[:, c, o0:o1],
                            start=False, stop=(c == NCHUNK - 1),
                            tile_position=(0, 0),
                        )
                    for (o0, o1) in slices:
                        nc.tensor.matmul(
                            pt[B:, o0:o1], a1[:, c, :], wt1[:, c, o0:o1],
                            start=False, stop=(c == NCHUNK - 1),
                            tile_position=(0, B),
                        )
                ob = opool.tile([2 * B, D], bf16, tag="ob")
                nc.vector.tensor_copy(ob[:], pt[:])
                eng_o = nc.scalar if j % 2 == 0 else nc.sync
                eng_o.dma_start(outp[j], ob[:])

            # Half-patch job: one patch, HD of its D outputs, runs last (its
            # inputs were prefetched before the pair loop) so the tail after
            # the final W byte is short.
            ph = pshpool.tile([B, HD], f32, tag="ph")
            nc.tensor.matmul(
                ph[:, :], ones2[:2, :B], tbh[:, :],
                start=True, stop=False, tile_position=(0, 0),
            )
            for c in range(NCHUNK):
                nc.tensor.matmul(
                    ph[:, :], ah[:, 0, c, :], wh[:, c, :],
                    start=False, stop=(c == NCHUNK - 1),
                    tile_position=(0, 0),
                )
            oh = opool.tile([B, HD], bf16, tag="oh")
            nc.vector.tensor_copy(oh[:], ph[:])
            nc.scalar.dma_start(outh[:, :], oh[:])

    nc.finalize()
    return nc


def _patchify(x):
    # [B, C, H, W] -> [B, 196, 768] in MAE ordering (n c h p w q -> n h w p q c)
    Bn, C, H, Wd = x.shape
    h = H // 16
    xr = x.reshape(Bn, C, h, 16, h, 16)
    xr = np.transpose(xr, (0, 2, 4, 3, 5, 1))
    return xr.reshape(Bn, h * h, 16 * 16 * C)


def _bias_hilo(v):
    hi = v.astype(ml_dtypes.bfloat16)
    lo = (v - hi.astype(np.float32)).astype(ml_dtypes.bfloat16)
    return np.ascontiguousarray(np.stack([hi, lo], axis=0))


def kernel(x, W, b, _trace=False, _tmpdir=None):
    global LAST_RESULTS

    x = np.asarray(x, dtype=np.float32)
    W = np.asarray(W, dtype=np.float32)
    b = np.asarray(b, dtype=np.float32)

    # Activations pre-divided by WSCALE (exact power-of-2) so that the fp8
    # weights can be stored as W*WSCALE, centering them in e3m4's narrow
    # exponent range; the products (x/s)(s*W) come out unscaled.
    patches = (_patchify(x) / WSCALE).astype(ml_dtypes.bfloat16)  # [64, 196, 768]
    Wb = (W * WSCALE).astype(ml_dtypes.float8_e3m4)               # [196, 768, 768]

    in_maps = []
    for k in range(N_CORES):
        idx = np.arange(k * FPC, (k + 1) * FPC)         # full patches
        hp = 8 * FPC + k // 2                           # shared half patch
        ho = (k % 2) * HD                               # its output offset

        psl = patches[:, list(idx) + [hp], :]           # [64, 25, 768]
        aT = np.ascontiguousarray(
            psl.transpose(2, 1, 0)                      # [768, 25, 64]
            .reshape(NCHUNK, 128, FPC + 1, B)
            .transpose(1, 2, 0, 3)                      # [128, 25, 6, 64]
        )
        wsl = Wb[idx]                                   # [24, 768, 768]
        Wt = np.ascontiguousarray(
            wsl.transpose(0, 2, 1)                      # [24, 768(i), 768(o)]
            .reshape(FPC, NCHUNK, 128, D)
            .transpose(0, 2, 1, 3)                      # [24, 128, 6, 768]
        )
        Wh = np.ascontiguousarray(
            Wb[hp, ho:ho + HD, :]                       # [384(o), 768(i)]
            .transpose(1, 0)                            # [768(i), 384(o)]
            .reshape(NCHUNK, 128, HD)
            .transpose(1, 0, 2)                         # [128, 6, 384]
        )
        hl = _bias_hilo(b[idx])                         # [2, 24, 768]
        bhl2 = np.ascontiguousarray(
            hl.transpose(1, 0, 2)                       # [24, 2, 768]
            .reshape(NPAIR, 4, D)                       # [hiA, loA, hiB, loB]
        )
        sel = np.zeros((4, 2 * B), dtype=ml_dtypes.bfloat16)
        sel[0:2, 0:B] = 1
        sel[2:4, B:2 * B] = 1
        in_maps.append({
            "aT": aT, "Wt": Wt, "Wh": Wh,
            "bhl2": bhl2, "bh": _bias_hilo(b[hp, ho:ho + HD]), "sel": sel,
        })

    if "F" not in _NC_CACHE:
        _NC_CACHE["F"] = _build()
    nc = _NC_CACHE["F"]

    res = run_bass_kernel_spmd(
        nc, in_maps, list(range(N_CORES)), trace=_trace, tmpdir=_tmpdir
    )
    LAST_RESULTS = res

    out = np.empty((B, N_CORES * FPC + 4, D), dtype=np.float32)
    for k in range(N_CORES):
        op = res.results[k]["outp"].astype(np.float32)  # [12, 128, 768]
        out[:, k * FPC:(k + 1) * FPC, :] = (
            op.reshape(FPC, B, D).transpose(1, 0, 2)
        )
        hp = 8 * FPC + k // 2
        ho = (k % 2) * HD
        out[:, hp, ho:ho + HD] = res.results[k]["outh"].astype(np.float32)
    return np.ascontiguousarray(out[:, :NP, :])


# revision 31
# speedup vs baseline: 1.1417x; 1.1417x over previous
"""Trainium2 Bass kernel for nn_NonsharedPatchEmbed_86827058856432.

Computes, for a patchified [64, 3, 224, 224] fp32 image batch,

    out[b, p, o] = sum_i patches[b, p, i] * W[p, o, i] + bias[p, o]

with 196 independent Linear(768->768) layers (one per patch).

The problem is HBM-bound on W traffic (196*768*768 elements, each used
once per core under patch sharding), so precision is spent where the bytes
are: W is stored as fp8 e3m4 (4 mantissa bits) pre-scaled by x64 to center
its distribution in e3m4's narrow exponent range, and the activations are
bf16 pre-divided by 64 (exact, so the products come out unscaled). The PE
accepts mixed bf16 x fp8 operands; PSUM accumulation stays fp32. Measured
end-to-end relative error is 1.30e-2 against the 2e-2 gate, deterministic
for the fixed harness inputs.

Distribution: 196 = 8 * 24.5, so each core gets 24 full patches plus ONE
HALF of a shared patch (384 of its 768 outputs): patches 0-191 go 24 per
core, and patches 192-195 are split into 8 output-halves, one per core.
Every core therefore reads exactly 24.5/196 of W -- perfect balance, and
the per-core DMA-engine pool (16 engines x ~22.4 GB/s ~= 360 GB/s) is the
roofline. The half-patch job runs LAST so the post-last-W drain (compute +
PSUM copy + output write) is ~4x smaller than a full pair's.

Per-core kernel (column-tiled pairs):
  - 12 pairs of full patches, then the half-patch job.
  - For each pair, patch A owns PSUM partitions 0-63 (tile_position (0, 0)),
    patch B owns partitions 64-127 ((0, 64)). Each streams its own W^T as
    the moving operand; the batch activations (aT chunks, [128 x 64]) are
    the stationary operand.
  - The bias is applied with ONE K=4 matmul per output slice: a host-built
    0/1 selector as the stationary operand routes [hiA, loA, hiB, loB]
    bf16 bias terms to the right PSUM partition halves, starting each
    accumulation group exactly (hi+lo reconstructs fp32 bias to ~1e-7).
  - Each patch's W rides ONE ring as a single 4608B-per-partition DMA
    (2304B packets measured ~11% slower per engine); pairs alternate
    patches across the two rings. Acts ride SP, outputs/bias ride ACT,
    keeping both rings byte-even (~9.7 MB each).
  - W dma_starts are issued first in each iteration so the big stream's
    descriptors stay at the queue heads.

Layouts per core:
  aT  [128, 25, 6, 64]  bf16  aT[i, p, c, b] = patches[b, patch(p), 128c+i]/64
                              (p = 0..23 full patches, p = 24 half patch)
  Wt  [24, 128, 6, 768] fp8   Wt[p, i, c, o] = 64*W[patch(p), o, 128c+i] (e3m4)
  Wh  [128, 6, 384]     fp8   half-patch W slice, same scaling
  bhl2 [12, 4, 768]     bf16  per pair: [hiA, loA, hiB, loB] bias terms
  bh  [2, 384]          bf16  half-patch bias hi + lo
  sel [4, 128]          bf16  bias selector: sel[k, m] = ((k < 2) == (m < 64))
  outp [12, 128, 768]   bf16  pair j rows 0-63 -> patch 2j, 64-127 -> 2j+1
  outh [64, 384]        bf16  half-patch outputs
"""

import sys
import types

import numpy as np
import ml_dtypes


def _ensure_ntff_hook():
    """Make ``antenv.axon_hooks`` importable and install the NTFF profile
    hook. The image's read-only ``antenv`` package lacks ``axon_hooks``, so
    ``trn_boot`` silently skips hook installation and
    ``run_bass_kernel_spmd(trace=True)`` would either crash on the import or
    skip tracing. Harmless no-op when the real module exists; any failure
    degrades to no-trace rather than an error."""
    try:
        from antenv.axon_hooks import get_axon_ntff_profile_hook  # noqa: F401
    except ImportError:
        mod = types.ModuleType("antenv.axon_hooks")
        _hook = [None]
        mod.set_axon_ntff_profile_hook = lambda h: _hook.__setitem__(0, h)
        mod.get_axon_ntff_profile_hook = lambda: _hook[0]
        sys.modules["antenv.axon_hooks"] = mod
        try:
            import antenv

            antenv.axon_hooks = mod
        except ImportError:
            pass
    try:
        import antenv.axon_hooks as ah

        if ah.get_axon_ntff_profile_hook() is None:
            from trn_agent_boot.trn_boot import _ntff_profile_via_ctypes

            ah.set_axon_ntff_profile_hook(
                _ntff_profile_via_ctypes("/opt/axon/libaxon_pjrt.so")
            )
    except Exception:
        pass


_ensure_ntff_hook()

import concourse.tile as tile
import concourse.mybir as mybir
from concourse import bacc
from concourse.bass_utils import run_bass_kernel_spmd

f32 = mybir.dt.float32
bf16 = mybir.dt.bfloat16
f8 = mybir.dt.float8e3   # e3m4: 4 mantissa bits
WSCALE = 64.0            # W stored as W*64 in fp8; act pre-divided by 64 (both exact)

N_CORES = 8
B = 64            # batch
D = 768           # in/out feature dim
HD = 384          # half of D (half-patch job width)
NP = 196          # real patches
FPC = 24          # full patches per core (8*24 = 192)
NPAIR = FPC // 2  # 12 pairs
NCHUNK = 6        # 768 / 128 contraction chunks

LAST_RESULTS = None    # BassKernelResults of the most recent run (for test.py)

_NC_CACHE = {}


def _build():
    nc = bacc.Bacc()
    aT = nc.declare_dram_parameter("aT", [128, FPC + 1, NCHUNK, B], bf16, isOutput=False)
    Wt = nc.declare_dram_parameter("Wt", [FPC, 128, NCHUNK, D], f8, isOutput=False)
    Wh = nc.declare_dram_parameter("Wh", [128, NCHUNK, HD], f8, isOutput=False)
    bhl2 = nc.declare_dram_parameter("bhl2", [NPAIR, 4, D], bf16, isOutput=False)
    bh = nc.declare_dram_parameter("bh", [2, HD], bf16, isOutput=False)
    sel = nc.declare_dram_parameter("sel", [4, 2 * B], bf16, isOutput=False)
    outp = nc.declare_dram_parameter("outp", [NPAIR, 2 * B, D], bf16, isOutput=True)
    outh = nc.declare_dram_parameter("outh", [B, HD], bf16, isOutput=True)

    with tile.TileContext(nc) as tc:
        with (
            tc.tile_pool(name="const", bufs=1) as cpool,
            tc.tile_pool(name="a", bufs=6) as apool,
            tc.tile_pool(name="w", bufs=10) as wpool,
            tc.tile_pool(name="o", bufs=4) as opool,
            tc.tile_pool(name="ps", bufs=3, space="PSUM") as pspool,
            tc.tile_pool(name="psh", bufs=1, space="PSUM") as pshpool,
        ):
            # Selector stationary for the pair bias matmul: one K=4 matmul
            # covers both column tiles exactly (hi+lo bf16 terms per patch).
            # ones2[k, m] = 1 iff (k < 2) == (m < 64); its top-left [2, 64]
            # corner doubles as the all-ones operand for the half-patch job.
            # (Built on the host: engine memsets can't start at partition 2.)
            ones2 = cpool.tile([4, 2 * B], bf16)

            slices = [(0, 512), (512, 768)]

            def wtile(p, eng):
                # Each patch's W rides ONE ring as a single 4608B-per-partition
                # transfer (2304B packets run ~11% slower per DMA engine);
                # pairs alternate patches across the two rings, which keeps
                # both byte-even. A single ring caps near ~200 GB/s, so the
                # alternation — not one ring — carries the dominant traffic.
                t = wpool.tile([128, NCHUNK, D], f8, tag="wt")
                eng.dma_start(t[:], Wt[p])
                return t

            pending_out = None
            for j in range(NPAIR):
                p0, p1 = 2 * j, 2 * j + 1
                wt0 = wtile(p0, nc.sync)
                wt1 = wtile(p1, nc.scalar)
                if pending_out is not None:
                    # Issue pair j-1's output write only AFTER pair j's W
                    # descriptors: an out dma_start waits on the PSUM-copy
                    # semaphore before ringing its doorbell, and the HWDGE
                    # engine is in-order — issuing it between W fetches
                    # throttles the W prefetch pipeline to ~one pair.
                    jj, obb = pending_out
                    eng_o = nc.scalar if jj % 2 == 0 else nc.sync
                    eng_o.dma_start(outp[jj], obb[:])
                if j == 0:
                    # Behind pair 0's W: the bias selector and the half-patch
                    # job's inputs. Prefetching the half job up front makes
                    # the final job start with everything resident (short
                    # drain); sequencing it after pair 0's W keeps the big
                    # stream's first descriptors at the queue heads.
                    nc.scalar.dma_start(ones2[:], sel[:, :])
                    wh = wpool.tile([128, NCHUNK, HD], f8, tag="wh")
                    nc.sync.dma_start(wh[:], Wh[:, :])
                    ah = apool.tile([128, 1, NCHUNK, B], bf16, tag="ah")
                    tbh = apool.tile([2, HD], bf16, tag="tbh")
                    nc.scalar.dma_start(ah[:], aT[:, FPC:FPC + 1])
                    nc.scalar.dma_start(tbh[:], bh[:, :])
                # Outputs exist only after compute, so they are inherently the
                # LAST bytes each queue moves; pinning them all to one ring
                # measured that ring finishing ~7.5us after the other.
                # Alternate outputs by pair parity (and acts oppositely, to
                # keep both rings byte-even) so the late tail splits evenly.
                at = apool.tile([128, 2, NCHUNK, B], bf16, tag="at")
                tb = apool.tile([4, D], bf16, tag="tb")
                eng_at = nc.sync if j % 2 == 0 else nc.scalar
                eng_at.dma_start(at[:], aT[:, p0:p0 + 2])
                nc.scalar.dma_start(tb[:], bhl2[j])
                a0 = at[:, 0]
                a1 = at[:, 1]

                pt = pspool.tile([2 * B, D], f32, tag="pt")
                for (o0, o1) in slices:
                    nc.tensor.matmul(
                        pt[:, o0:o1], ones2[:], tb[:, o0:o1],
                        start=True, stop=False,
                    )
                for c in range(NCHUNK):
                    # A's two output slices adjacent so the stationary
                    # operand only changes once per chunk per patch.
                    for (o0, o1) in slices:
                        nc.tensor.matmul(
                            pt[:B, o0:o1], a0[:, c, :], wt0[:, c, o0:o1],
                            start=False, stop=(c == NCHUNK - 1),
                            tile_position=(0, 0),
                        )
                    for (o0, o1) in slices:
                        nc.tensor.matmul(
                            pt[B:, o0:o1], a1[:, c, :], wt1[:, c, o0:o1],
                            start=False, stop=(c == NCHUNK - 1),
                            tile_position=(0, B),
                        )
                ob = opool.tile([2 * B, D], bf16, tag="ob")
                nc.vector.tensor_copy(ob[:], pt[:])
                pending_out = (j, ob)

            # Flush the last pair's output before the half-patch compute.
            jj, obb = pending_out
            eng_o = nc.scalar if jj % 2 == 0 else nc.sync
            eng_o.dma_start(outp[jj], obb[:])

            # Half-patch job: one patch, HD of its D outputs, runs last (its
            # inputs were prefetched before the pair loop) so the tail after
            # the final W byte is short.
            ph = pshpool.tile([B, HD], f32, tag="ph")
            nc.tensor.matmul(
                ph[:, :], ones2[:2, :B], tbh[:, :],
                start=True, stop=False, tile_position=(0, 0),
            )
            for c in range(NCHUNK):
                nc.tensor.matmul(
                    ph[:, :], ah[:, 0, c, :], wh[:, c, :],
                    start=False, stop=(c == NCHUNK - 1),
                    tile_position=(0, 0),
                )
            oh = opool.tile([B, HD], bf16, tag="oh")
            nc.vector.tensor_copy(oh[:], ph[:])
            nc.scalar.dma_start(outh[:, :], oh[:])

    nc.finalize()
    return nc


def _patchify(x):
    # [B, C, H, W] -> [B, 196, 768] in MAE ordering (n c h p w q -> n h w p q c)
    Bn, C, H, Wd = x.shape
    h = H // 16
    xr = x.reshape(Bn, C, h, 16, h, 16)
    xr = np.transpose(xr, (0, 2, 4, 3, 5, 1))
    return xr.reshape(Bn, h * h, 16 * 16 * C)


def _bias_hilo(v):
    hi = v.astype(ml_dtypes.bfloat16)
    lo = (v - hi.astype(np.float32)).astype(ml_dtypes.bfloat16)
    return np.ascontiguousarray(np.stack([hi, lo], axis=0))


def kernel(x, W, b, _trace=False, _tmpdir=None):
    global LAST_RESULTS

    x = np.asarray(x, dtype=np.float32)
    W = np.asarray(W, dtype=np.float32)
    b = np.asarray(b, dtype=np.float32)

    # Activations pre-divided by WSCALE (exact power-of-2) so that the fp8
    # weights can be stored as W*WSCALE, centering them in e3m4's narrow
    # exponent range; the products (x/s)(s*W) come out unscaled.
    patches = (_patchify(x) / WSCALE).astype(ml_dtypes.bfloat16)  # [64, 196, 768]
    Wb = (W * WSCALE).astype(ml_dtypes.float8_e3m4)               # [196, 768, 768]

    in_maps = []
    for k in range(N_CORES):
        idx = np.arange(k * FPC, (k + 1) * FPC)         # full patches
        hp = 8 * FPC + k // 2                           # shared half patch
        ho = (k % 2) * HD                               # its output offset

        psl = patches[:, list(idx) + [hp], :]           # [64, 25, 768]
        aT = np.ascontiguousarray(
            psl.transpose(2, 1, 0)                      # [768, 25, 64]
            .reshape(NCHUNK, 128, FPC + 1, B)
            .transpose(1, 2, 0, 3)                      # [128, 25, 6, 64]
        )
        wsl = Wb[idx]                                   # [24, 768, 768]
        Wt = np.ascontiguousarray(
            wsl.transpose(0, 2, 1)                      # [24, 768(i), 768(o)]
            .reshape(FPC, NCHUNK, 128, D)
            .transpose(0, 2, 1, 3)                      # [24, 128, 6, 768]
        )
        Wh = np.ascontiguousarray(
            Wb[hp, ho:ho + HD, :]                       # [384(o), 768(i)]
            .transpose(1, 0)                            # [768(i), 384(o)]
            .reshape(NCHUNK, 128, HD)
            .transpose(1, 0, 2)                         # [128, 6, 384]
        )
        hl = _bias_hilo(b[idx])                         # [2, 24, 768]
        bhl2 = np.ascontiguousarray(
            hl.transpose(1, 0, 2)                       # [24, 2, 768]
            .reshape(NPAIR, 4, D)                       # [hiA, loA, hiB, loB]
        )
        sel = np.zeros((4, 2 * B), dtype=ml_dtypes.bfloat16)
        sel[0:2, 0:B] = 1
        sel[2:4, B:2 * B] = 1
        in_maps.append({
            "aT": aT, "Wt": Wt, "Wh": Wh,
            "bhl2": bhl2, "bh": _bias_hilo(b[hp, ho:ho + HD]), "sel": sel,
        })

    if "F" not in _NC_CACHE:
        _NC_CACHE["F"] = _build()
    nc = _NC_CACHE["F"]

    res = run_bass_kernel_spmd(
        nc, in_maps, list(range(N_CORES)), trace=_trace, tmpdir=_tmpdir
    )
    LAST_RESULTS = res

    out = np.empty((B, N_CORES * FPC + 4, D), dtype=np.float32)
    for k in range(N_CORES):
        op = res.results[k]["outp"].astype(np.float32)  # [12, 128, 768]
        out[:, k * FPC:(k + 1) * FPC, :] = (
            op.reshape(FPC, B, D).transpose(1, 0, 2)
        )
        hp = 8 * FPC + k // 2
        ho = (k % 2) * HD
        out[:, hp, ho:ho + HD] = res.results[k]["outh"].astype(np.float32)
    return np.ascontiguousarray(out[:, :NP, :])
